# revision 9
# baseline (speedup 1.0000x reference)
"""Multi-head self-attention with RoPE on 8 Trainium2 NeuronCores.

Problem: B=2, S=2048, D_MODEL=2048, 16 heads x d_k=128, causal, RoPE on Q/K.

Sharding (hardcoded): core c -> batch b=c//4, head group g=c%4 (heads 4g..4g+3).
Data parallel on batch, tensor parallel on heads; q/k/v projections column-
sharded, output projection row-sharded with the partial sums reduced on host.

Device kernel v3 (identical program on all 8 cores, different data):
  All matmul operands are bfloat16 (PSUM accumulation stays fp32) — same PE
  streaming rate as float32r but enables Fast Weight Load on every stationary
  and halves HBM+SBUF traffic.

  DMA: x and the q/k/v weights are relaid out on the host so every transfer
  is a (128, 1024..2048) contiguous row-block — ~100 descriptors total
  instead of 400+ (the Sync engine issues descriptors serially at ~0.6us
  each, which was the v2 bottleneck at phase boundaries).  The pair-0 q/k
  weights stream in behind the first V-pass chunk so the projection phase
  starts with zero DMA wait.

  Phase schedule, chosen so the PE instruction stream never waits on a
  just-issued Vector/Scalar op:
    1. V pass          (x-subtile stationary, wv moving)
    2. proj pair 0     (w-subtile stationary, x moving); RoPE per q-chunk on
                       Vector runs behind the next q-chunk's matmuls
                       (PSUM accumulators alternate banks t0-3/t4-7 per qc)
    3. attn pair 0     interleaved with proj pair 1 as PE filler
    4. attn pair 1     interleaved with output projection as PE filler
  Attention per (head, q-chunk): S^T = KT.T @ QT per 128-row k-block, exp on
  Scalar, the 128-wide causal boundary block multiplied on GpSimd, den =
  ones(128x128).T @ P~T (broadcast denominator for free), AV = V.T @ P~T;
  1/den via the fast approximate DVE reciprocal.  Fully-masked k-blocks are
  skipped and diagonal blocks only stream live columns (512/384/256/128).
  RoPE even/odd interleave is pre-permuted into wq/wk rows on host.
"""

import sys

sys.path.insert(0, "/opt/trn_rl_repo")

import math

import ml_dtypes
import numpy as np

import concourse.bass as bass
import concourse.mybir as mybir
import concourse.tile as tile
from concourse import bacc
from concourse.bass_utils import run_bass_kernel_spmd

f32 = mybir.dt.float32
bf16 = mybir.dt.bfloat16

B = 2
S = 2048
D = 2048
H = 16
DK = 128
H_CORE = 4  # heads per core
DL = H_CORE * DK  # local feature dim 512
ET = D // 128  # 16 e-tiles (contraction over d_model)
GT = 4  # e-tile groups of 4 (DMA batching)
QC = S // 512  # 4 q-chunks
THETA = 10000.0
SCALE = 1.0 / math.sqrt(DK)
LAG = 3  # scores run LAG k-blocks ahead of den/AV

N_CORES = 8


def _build():
    nc = bacc.Bacc("TRN2", target_bir_lowering=False, debug=False)

    # xB[(qc*4+g)*128+p, e*512+c] = x^T[(4g+e)*128+p, qc*512+c]
    xB_d = nc.dram_tensor("xB", [QC * GT * 128, 4 * 512], bf16,
                          kind="ExternalInput")
    # wqB[(pair*4+g)*128+p, e*256+c] = wq^T[(4g+e)*128+p, pair*256+c]
    wqB_d = nc.dram_tensor("wqB", [2 * GT * 128, 4 * 256], bf16,
                           kind="ExternalInput")
    wkB_d = nc.dram_tensor("wkB", [2 * GT * 128, 4 * 256], bf16,
                           kind="ExternalInput")
    # wvB[g*128+p, e*512+c] = wv^T[(4g+e)*128+p, c]
    wvB_d = nc.dram_tensor("wvB", [GT * 128, 4 * 512], bf16,
                           kind="ExternalInput")
    woT_d = nc.dram_tensor("woT", [DL, D], bf16, kind="ExternalInput")
    cosT_d = nc.dram_tensor("cosT", [64, S], f32, kind="ExternalInput")
    sinT_d = nc.dram_tensor("sinT", [64, S], f32, kind="ExternalInput")
    cmask_d = nc.dram_tensor("cmask", [4, 128, 512], bf16, kind="ExternalInput")
    outT_d = nc.dram_tensor("outT", [D, S], f32, kind="ExternalOutput")

    Exp = mybir.ActivationFunctionType.Exp

    with tile.TileContext(nc) as tc:
      with tc.tile_pool(name="const", bufs=1) as const, \
           tc.tile_pool(name="persist", bufs=1) as persist, \
           tc.tile_pool(name="wqp", bufs=8) as wqp, \
           tc.tile_pool(name="wkp", bufs=8) as wkp, \
           tc.tile_pool(name="xsp", bufs=8) as xsp, \
           tc.tile_pool(name="ropet", bufs=1) as ropet, \
           tc.tile_pool(name="ptp", bufs=6) as ptp, \
           tc.tile_pool(name="smallp", bufs=2) as smallp, \
           tc.tile_pool(name="psum", bufs=1, space="PSUM") as psum:

        V = [persist.tile([128, DL], bf16, tag=f"v{st}", name=f"v{st}")
             for st in range(ET)]
        QT = [[persist.tile([DK, S], bf16, tag=f"qt{p}_{i}", name=f"qt{p}_{i}")
               for i in range(2)] for p in range(2)]
        KT = [[persist.tile([DK, S], bf16, tag=f"kt{p}_{i}", name=f"kt{p}_{i}")
               for i in range(2)] for p in range(2)]
        OT = [persist.tile([DK, S], bf16, tag=f"ot{h}", name=f"ot{h}")
              for h in range(H_CORE)]

        def load_wqk(p):
            wq_sb, wk_sb = [], []
            for g in range(GT):
                wqt = wqp.tile([128, 1024], bf16, tag="wq", name="wq")
                nc.sync.dma_start(
                    wqt[:], wqB_d[(p * GT + g) * 128 : (p * GT + g + 1) * 128, :]
                )
                wq_sb.append(wqt)
                wkt = wkp.tile([128, 1024], bf16, tag="wk", name="wk")
                nc.sync.dma_start(
                    wkt[:], wkB_d[(p * GT + g) * 128 : (p * GT + g + 1) * 128, :]
                )
                wk_sb.append(wkt)
            return wq_sb, wk_sb

        # ---- PE warm-up: dummy matmuls while the first DMAs are in flight
        # (HAM un-throttles after ~3.4us of PE activity; this moves that
        # ramp into the otherwise-idle DMA startup window)
        warm = const.tile([128, 128], bf16, tag="warm")
        nc.vector.memset(warm[:], 0.0)
        wacc = psum.tile([128, 128], f32, tag="t7", name="wacc")
        for _ in range(40):
            nc.tensor.matmul(wacc[:], warm[:], warm[:], start=True, stop=True)

        # ---- V pass: x-subtile stationary, wv moving -------------------
        # qc0: wv and x interleave so the first matmul starts ~3us in; the
        # pair-0 q/k weights queue behind qc1's x and land mid-phase.
        with tc.tile_pool(name="wvp", bufs=4) as wvp:
            wv_sb = []
            wqk0 = None
            for qc in range(QC):
                tb = 0 if qc % 2 == 0 else 4
                vacc = [
                    psum.tile([128, DL], f32, tag=f"t{tb + i}", name=f"vacc{i}")
                    for i in range(4)
                ]
                xts = []
                for g in range(GT):
                    if qc == 0:
                        wt = wvp.tile([128, 4 * 512], bf16, tag="wv", name="wv")
                        if g == 0:
                            # split so the e=0 slices land first and the
                            # first matmul starts ~1.4us in
                            nc.sync.dma_start(
                                wt[:, 0:512], wvB_d[0:128, 0:512]
                            )
                            nc.sync.dma_start(
                                wt[:, 512:2048], wvB_d[0:128, 512:2048]
                            )
                        else:
                            nc.sync.dma_start(
                                wt[:], wvB_d[g * 128 : (g + 1) * 128, :]
                            )
                        wv_sb.append(wt)
                    xt = xsp.tile([128, 4 * 512], bf16, tag="xs", name="xs")
                    if qc == 0 and g == 0:
                        nc.sync.dma_start(xt[:, 0:512], xB_d[0:128, 0:512])
                        nc.sync.dma_start(
                            xt[:, 512:2048], xB_d[0:128, 512:2048]
                        )
                    else:
                        nc.sync.dma_start(
                            xt[:],
                            xB_d[(qc * GT + g) * 128 : (qc * GT + g + 1) * 128, :],
                        )
                    xts.append(xt)
                if qc == 1:
                    # pair-0 q/k weights: queue behind qc1's x so they don't
                    # delay it; they land well before the projection phase
                    wqk0 = load_wqk(0)
                for g in range(GT):
                    for e in range(4):
                        et = 4 * g + e
                        es = slice(e * 512, (e + 1) * 512)
                        for sl in range(4):
                            nc.tensor.matmul(
                                vacc[sl][:],
                                xts[g][:, e * 512 + sl * 128 : e * 512 + (sl + 1) * 128],
                                wv_sb[g][:, es],
                                start=(et == 0),
                                stop=(et == ET - 1),
                            )
                for sl in range(4):
                    nc.any.tensor_copy(V[qc * 4 + sl][:], vacc[sl][:])

        # ---- constants (used from proj p0's RoPE on) -------------------
        cos2 = const.tile([128, S], f32, tag="cos2")
        sin2 = const.tile([128, S], f32, tag="sin2")
        nc.sync.dma_start(cos2[0:64, :], cosT_d[:, :])
        nc.sync.dma_start(cos2[64:128, :], cosT_d[:, :])
        nc.sync.dma_start(sin2[0:64, :], sinT_d[:, :])
        nc.sync.dma_start(sin2[64:128, :], sinT_d[:, :])
        # only the 128-wide causal boundary block of each diagonal is ever
        # partially masked; the rest is all-ones (right of it) or skipped
        masks = []
        for j in range(4):
            mt = const.tile([128, 128], bf16, tag=f"mask{j}", name=f"mask{j}")
            nc.sync.dma_start(mt[:], cmask_d[j, :, 128 * j : 128 * (j + 1)])
            masks.append(mt)
        ones_bf = const.tile([128, 128], bf16, tag="ones_bf")
        nc.vector.memset(ones_bf[:], 1.0)

        def rope(dst, ev, od, qs):
            """ev/od: PSUM accumulators (128,512), rows [hA;hB]; dst bf16."""
            c = cos2[:, qs]
            sn = sin2[:, qs]
            m1 = ropet.tile([128, 512], f32, tag="m1")
            m2 = ropet.tile([128, 512], f32, tag="m2")
            n1 = ropet.tile([128, 512], f32, tag="n1")
            n2 = ropet.tile([128, 512], f32, tag="n2")
            nc.vector.tensor_mul(m1[:], ev[:], c)
            nc.vector.tensor_mul(m2[:], od[:], sn)
            nc.vector.tensor_mul(n1[:], ev[:], sn)
            nc.vector.tensor_mul(n2[:], od[:], c)
            nc.vector.tensor_sub(dst[0][0:64, qs], m1[0:64, :], m2[0:64, :])
            nc.vector.tensor_sub(dst[1][0:64, qs], m1[64:128, :], m2[64:128, :])
            nc.vector.tensor_add(dst[0][64:128, qs], n1[0:64, :], n2[0:64, :])
            nc.vector.tensor_add(
                dst[1][64:128, qs], n1[64:128, :], n2[64:128, :]
            )

        # ---- proj pair 0: PSUM banks alternate per qc parity -----------
        for qc in range(QC):
            tb = 0 if qc % 2 == 0 else 4
            qs = slice(qc * 512, (qc + 1) * 512)
            qe = psum.tile([128, 512], f32, tag=f"t{tb}", name="qe")
            qo = psum.tile([128, 512], f32, tag=f"t{tb+1}", name="qo")
            ke = psum.tile([128, 512], f32, tag=f"t{tb+2}", name="ke")
            ko = psum.tile([128, 512], f32, tag=f"t{tb+3}", name="ko")
            for g in range(GT):
                xt = xsp.tile([128, 4 * 512], bf16, tag="xs", name="xs")
                nc.sync.dma_start(
                    xt[:],
                    xB_d[(qc * GT + g) * 128 : (qc * GT + g + 1) * 128, :],
                )
                for e in range(4):
                    et = 4 * g + e
                    es = slice(e * 512, (e + 1) * 512)
                    st_, sp_ = et == 0, et == ET - 1
                    nc.tensor.matmul(
                        qe[:], wqk0[0][g][:, e * 256 : e * 256 + 128],
                        xt[:, es], start=st_, stop=sp_,
                    )
                    nc.tensor.matmul(
                        qo[:], wqk0[0][g][:, e * 256 + 128 : e * 256 + 256],
                        xt[:, es], start=st_, stop=sp_,
                    )
                    nc.tensor.matmul(
                        ke[:], wqk0[1][g][:, e * 256 : e * 256 + 128],
                        xt[:, es], start=st_, stop=sp_,
                    )
                    nc.tensor.matmul(
                        ko[:], wqk0[1][g][:, e * 256 + 128 : e * 256 + 256],
                        xt[:, es], start=st_, stop=sp_,
                    )
            rope(QT[0], qe, qo, qs)
            rope(KT[0], ke, ko, qs)

        def attn_group(p, hi, qc, feed):
            """Attention for (pair p, head-in-pair hi, q-chunk qc).  `feed`
            emits filler PE work between k-blocks."""
            hh = 2 * p + hi
            qs = slice(qc * 512, (qc + 1) * 512)
            nkt = 4 * qc + 4
            den = psum.tile([128, 512], f32, tag="t6", name="den")
            oacc = psum.tile([128, 512], f32, tag="t7", name="oacc")
            pts = {}

            def consume(kt):
                j = kt - 4 * qc
                o = 128 * j if j > 0 else 0
                cs = slice(o, 512)
                pt = pts.pop(kt)
                nc.tensor.matmul(
                    den[:, cs], ones_bf[:], pt[:, cs],
                    start=(kt == 0), stop=(kt == nkt - 1),
                    skip_group_check=True,
                )
                nc.tensor.matmul(
                    oacc[:, cs],
                    V[kt][:, hh * 128 : (hh + 1) * 128],
                    pt[:, cs],
                    start=(kt == 0),
                    stop=(kt == nkt - 1),
                    skip_group_check=True,
                )

            for kt in range(nkt):
                j = kt - 4 * qc
                o = 128 * j if j > 0 else 0
                cs = slice(o, 512)
                sps = psum.tile(
                    [128, 512], f32,
                    tag=("t4" if kt % 2 == 0 else "t5"),
                    name="sps",
                )
                nc.tensor.matmul(
                    sps[:, cs],
                    KT[p][hi][:, kt * 128 : (kt + 1) * 128],
                    QT[p][hi][:, qc * 512 + o : (qc + 1) * 512],
                    start=True,
                    stop=True,
                )
                pt = ptp.tile([128, 512], bf16, tag="pt")
                nc.scalar.activation(pt[:, cs], sps[:, cs], Exp, scale=SCALE)
                if j >= 0:
                    jb = slice(128 * j, 128 * (j + 1))
                    nc.gpsimd.tensor_mul(pt[:, jb], pt[:, jb], masks[j][:])
                pts[kt] = pt
                if kt >= LAG:
                    consume(kt - LAG)
                feed()
            for kt in range(max(0, nkt - LAG), nkt):
                consume(kt)
            rec = smallp.tile([128, 512], f32, tag="rec")
            nc.vector.reciprocal_approx_fast(rec[:], den[:])
            nc.vector.tensor_mul(OT[hh][:, qs], oacc[:], rec[:])

        class Filler:
            """Emit queued PE work items between attn blocks, paced so the
            queue drains evenly over the expected number of feed calls."""

            def __init__(self, items, expected_feeds):
                self.items = list(items)
                self.pos = 0
                self.feeds_left = max(1, expected_feeds)

            def feed(self):
                left = len(self.items) - self.pos
                n = -(-left // self.feeds_left)  # ceil
                for _ in range(min(n, left)):
                    self.items[self.pos]()
                    self.pos += 1
                if self.feeds_left > 1:
                    self.feeds_left -= 1

            def drain(self):
                while self.pos < len(self.items):
                    self.items[self.pos]()
                    self.pos += 1

        # ---- attn pair 0, proj pair 1 as filler ------------------------
        wqk1 = load_wqk(1)
        # x DMAs for pair 1 are prefetched one g-group ahead of their use
        p1_xts = {}

        def p1_dma(qc, g):
            xt = xsp.tile([128, 4 * 512], bf16, tag="xs", name="xs")
            nc.sync.dma_start(
                xt[:],
                xB_d[(qc * GT + g) * 128 : (qc * GT + g + 1) * 128, :],
            )
            p1_xts[(qc, g)] = xt

        p1_dma(0, 0)
        p1_items = []
        for qc in range(QC):
            qs = slice(qc * 512, (qc + 1) * 512)
            qe = psum.tile([128, 512], f32, tag="t0", name="qe1")
            qo = psum.tile([128, 512], f32, tag="t1", name="qo1")
            ke = psum.tile([128, 512], f32, tag="t2", name="ke1")
            ko = psum.tile([128, 512], f32, tag="t3", name="ko1")

            def mk_e(qc, qe, qo, ke, ko, g, e):
                def run():
                    if e == 0:
                        nxt = (qc * GT + g) + 1  # prefetch next g-group
                        if nxt < QC * GT:
                            p1_dma(nxt // GT, nxt % GT)
                    xt = p1_xts[(qc, g)]
                    et = 4 * g + e
                    es = slice(e * 512, (e + 1) * 512)
                    st_, sp_ = et == 0, et == ET - 1
                    nc.tensor.matmul(
                        qe[:], wqk1[0][g][:, e * 256 : e * 256 + 128],
                        xt[:, es], start=st_, stop=sp_,
                    )
                    nc.tensor.matmul(
                        qo[:], wqk1[0][g][:, e * 256 + 128 : e * 256 + 256],
                        xt[:, es], start=st_, stop=sp_,
                    )
                    nc.tensor.matmul(
                        ke[:], wqk1[1][g][:, e * 256 : e * 256 + 128],
                        xt[:, es], start=st_, stop=sp_,
                    )
                    nc.tensor.matmul(
                        ko[:], wqk1[1][g][:, e * 256 + 128 : e * 256 + 256],
                        xt[:, es], start=st_, stop=sp_,
                    )

                return run

            for g in range(GT):
                for e in range(4):
                    p1_items.append(mk_e(qc, qe, qo, ke, ko, g, e))

            def mk_rope(qs, qe, qo, ke, ko):
                def run():
                    rope(QT[1], qe, qo, qs)
                    rope(KT[1], ke, ko, qs)

                return run

            p1_items.append(mk_rope(qs, qe, qo, ke, ko))

        f1 = Filler(p1_items, expected_feeds=80)
        for qc in range(QC):
            for hi in range(2):
                attn_group(0, hi, qc, f1.feed)
        f1.drain()

        # ---- attn pair 1, output projection as filler ------------------
        with tc.tile_pool(name="wop", bufs=4) as wop, \
             tc.tile_pool(name="stg", bufs=8) as stg:
            wo_sb = []
            for hh in range(H_CORE):
                wt = wop.tile([128, D], bf16, tag="wo", name="wo")
                nc.sync.dma_start(wt[:], woT_d[hh * 128 : (hh + 1) * 128, :])
                wo_sb.append(wt)

            def mk_out(qc, et):
                qs = slice(qc * 512, (qc + 1) * 512)

                def run():
                    facc = psum.tile(
                        [128, 512], f32, tag=f"t{(qc * ET + et) % 4}",
                        name="facc",
                    )
                    for hh in range(H_CORE):
                        nc.tensor.matmul(
                            facc[:],
                            wo_sb[hh][:, et * 128 : (et + 1) * 128],
                            OT[hh][:, qs],
                            start=(hh == 0),
                            stop=(hh == H_CORE - 1),
                        )
                    st = stg.tile([128, 512], f32, tag="stg")
                    nc.any.tensor_copy(st[:], facc[:])
                    nc.sync.dma_start(
                        outT_d[et * 128 : (et + 1) * 128, qs], st[:]
                    )

                return run

            # out-proj items for q-chunk qc become runnable once attn pair 1
            # finishes that q-chunk; feed them as filler into the next one.
            fillers = [
                Filler([mk_out(qc, et) for et in range(ET)],
                       expected_feeds=2 * (4 * (qc + 1) + 4))
                for qc in range(QC)
            ]
            for qc in range(QC):
                feed = fillers[qc - 1].feed if qc > 0 else (lambda: None)
                for hi in range(2):
                    attn_group(1, hi, qc, feed)
            for f in fillers:
                f.drain()

    return nc


_NC = None


def _get_nc():
    global _NC
    if _NC is None:
        _NC = _build()
        _NC.compile()
    return _NC


def _rope_perm_rows():
    """Row permutation applied to wq/wk for one core's 4 heads.

    Per head-pair p: [hA even dims, hB even dims, hA odd dims, hB odd dims]
    so the device sees even/odd deinterleaved, pair-stacked projections.
    Returns indices into the local (4*DK,) head-row block.
    """
    idx = []
    for p in range(2):
        ha, hb = 2 * p, 2 * p + 1
        idx.extend(ha * DK + np.arange(0, DK, 2))
        idx.extend(hb * DK + np.arange(0, DK, 2))
        idx.extend(ha * DK + np.arange(1, DK, 2))
        idx.extend(hb * DK + np.arange(1, DK, 2))
    return np.asarray(idx)


def _host_tables(positions):
    """cos/sin tables (64, S) float32, matching the fp32 reference math."""
    dim_idx = np.arange(0, DK, 2, dtype=np.float32)
    freqs = np.float32(THETA) ** (dim_idx / np.float32(DK))
    angles = positions.astype(np.float32)[:, None] / freqs[None, :]  # (S, 64)
    return (
        np.ascontiguousarray(np.cos(angles).T.astype(np.float32)),
        np.ascontiguousarray(np.sin(angles).T.astype(np.float32)),
    )


def _causal_masks():
    m = np.zeros((4, 128, 512), dtype=np.float32)
    p = np.arange(128)[:, None]
    f = np.arange(512)[None, :]
    for j in range(4):
        m[j] = (128 * j + p <= f).astype(np.float32)
    return m


def _bf(a):
    return np.ascontiguousarray(a).astype(ml_dtypes.bfloat16)


def _batch_rows(aT, cols_per_e):
    """(ET*128, W) -> (GT*128, 4*W'): pack 4 consecutive e-tiles side by side
    in the free dim so one row-block DMA carries four contraction tiles.

    aT[(4g+e)*128+p, c] -> out[g*128+p, e*W'+c]  (W' = cols_per_e slice width)
    """
    w = aT.shape[1]
    a = aT.reshape(GT, 4, 128, w)  # [g, e, p, c]
    return a.transpose(0, 2, 1, 3).reshape(GT * 128, 4 * w)


def _make_in_maps(inputs):
    x = np.asarray(inputs["x"], dtype=np.float32)
    wq = np.asarray(inputs["wq"], dtype=np.float32)
    wk = np.asarray(inputs["wk"], dtype=np.float32)
    wv = np.asarray(inputs["wv"], dtype=np.float32)
    wo = np.asarray(inputs["wo"], dtype=np.float32)
    token_positions = np.asarray(inputs["token_positions"])

    perm = _rope_perm_rows()
    cmask = _bf(_causal_masks())

    in_maps = []
    for c in range(N_CORES):
        b = c // 4
        g = c % 4
        rows = slice(g * DL, (g + 1) * DL)
        cosT, sinT = _host_tables(token_positions[b])

        xT = x[b].T  # (D, S)
        # xB[(qc*4+g)*128+p, e*512+c] = xT[(4g+e)*128+p, qc*512+c]
        xa = xT.reshape(GT, 4, 128, QC, 512)  # [g, e, p, qc, c]
        xB = xa.transpose(3, 0, 2, 1, 4).reshape(QC * GT * 128, 4 * 512)

        wqT = wq[rows][perm].T  # (D, DL)
        wkT = wk[rows][perm].T
        # wB[(pair*4+g)*128+p, e*256+c] = wT[(4g+e)*128+p, pair*256+c]
        def wqk_batch(wT):
            a = wT.reshape(GT, 4, 128, 2, 256)  # [g, e, p, pair, c]
            return a.transpose(3, 0, 2, 1, 4).reshape(2 * GT * 128, 4 * 256)

        in_maps.append(
            {
                "xB": _bf(xB),
                "wqB": _bf(wqk_batch(wqT)),
                "wkB": _bf(wqk_batch(wkT)),
                "wvB": _bf(_batch_rows(wv[rows].T, 512)),
                "woT": _bf(wo[:, rows].T),
                "cosT": cosT,
                "sinT": sinT,
                "cmask": cmask,
            }
        )
    return in_maps


def kernel(x, wq, wk, wv, wo, token_positions):
    nc = _get_nc()
    in_maps = _make_in_maps(
        {
            "x": x,
            "wq": wq,
            "wk": wk,
            "wv": wv,
            "wo": wo,
            "token_positions": token_positions,
        }
    )
    res = run_bass_kernel_spmd(nc, in_maps, list(range(N_CORES)))

    out = np.zeros((B, S, D), dtype=np.float32)
    for c in range(N_CORES):
        out[c // 4] += res.results[c]["outT"].T
    return out


# revision 10
# speedup vs baseline: 1.1898x; 1.1898x over previous
"""Multi-head self-attention with RoPE on 8 Trainium2 NeuronCores.

Problem: B=2, S=2048, D_MODEL=2048, 16 heads x d_k=128, causal, RoPE on Q/K.

Sharding (hardcoded): core c -> batch b=c//4, head group g=c%4 (heads 4g..4g+3).
Data parallel on batch, tensor parallel on heads; q/k/v projections column-
sharded, output projection row-sharded with the partial sums reduced on host.

Device kernel v3 (identical program on all 8 cores, different data):
  All matmul operands are bfloat16 (PSUM accumulation stays fp32) — same PE
  streaming rate as float32r but enables Fast Weight Load on every stationary
  and halves HBM+SBUF traffic.

  DMA: x and the q/k/v weights are relaid out on the host so every transfer
  is a (128, 1024..2048) contiguous row-block — ~100 descriptors total
  instead of 400+ (the Sync engine issues descriptors serially at ~0.6us
  each, which was the v2 bottleneck at phase boundaries).  The pair-0 q/k
  weights stream in behind the first V-pass chunk so the projection phase
  starts with zero DMA wait.

  Phase schedule, chosen so the PE instruction stream never waits on a
  just-issued Vector/Scalar op:
    1. V pass          (x-subtile stationary, wv moving)
    2. proj pair 0     (w-subtile stationary, x moving); RoPE per q-chunk on
                       Vector runs behind the next q-chunk's matmuls
                       (PSUM accumulators alternate banks t0-3/t4-7 per qc)
    3. attn pair 0     interleaved with proj pair 1 as PE filler
    4. attn pair 1     interleaved with output projection as PE filler
  Attention per (head, q-chunk): S^T = KT.T @ QT per 128-row k-block, exp on
  Scalar, the 128-wide causal boundary block multiplied on GpSimd, den =
  ones(128x128).T @ P~T (broadcast denominator for free), AV = V.T @ P~T;
  1/den via the fast approximate DVE reciprocal.  Fully-masked k-blocks are
  skipped and diagonal blocks only stream live columns (512/384/256/128).
  RoPE even/odd interleave is pre-permuted into wq/wk rows on host.
"""

import sys

sys.path.insert(0, "/opt/trn_rl_repo")

import math

import ml_dtypes
import numpy as np

import concourse.bass as bass
import concourse.mybir as mybir
import concourse.tile as tile
from concourse import bacc
from concourse.bass_utils import run_bass_kernel_spmd

f32 = mybir.dt.float32
bf16 = mybir.dt.bfloat16

B = 2
S = 2048
D = 2048
H = 16
DK = 128
H_CORE = 4  # heads per core
DL = H_CORE * DK  # local feature dim 512
ET = D // 128  # 16 e-tiles (contraction over d_model)
GT = 4  # e-tile groups of 4 (DMA batching)
QC = S // 512  # 4 q-chunks
THETA = 10000.0
SCALE = 1.0 / math.sqrt(DK)
LAG = 3  # scores run LAG k-blocks ahead of den/AV

N_CORES = 8


def _build():
    nc = bacc.Bacc("TRN2", target_bir_lowering=False, debug=False)

    # xB[(qc*4+g)*128+p, e*512+c] = x^T[(4g+e)*128+p, qc*512+c]
    xB_d = nc.dram_tensor("xB", [QC * GT * 128, 4 * 512], bf16,
                          kind="ExternalInput")
    # wqB[(pair*4+g)*128+p, e*256+c] = wq^T[(4g+e)*128+p, pair*256+c]
    wqB_d = nc.dram_tensor("wqB", [2 * GT * 128, 4 * 256], bf16,
                           kind="ExternalInput")
    wkB_d = nc.dram_tensor("wkB", [2 * GT * 128, 4 * 256], bf16,
                           kind="ExternalInput")
    # wvB[g*128+p, e*512+c] = wv^T[(4g+e)*128+p, c]
    wvB_d = nc.dram_tensor("wvB", [GT * 128, 4 * 512], bf16,
                           kind="ExternalInput")
    woT_d = nc.dram_tensor("woT", [DL, D], bf16, kind="ExternalInput")
    cosT_d = nc.dram_tensor("cosT", [64, S], f32, kind="ExternalInput")
    sinT_d = nc.dram_tensor("sinT", [64, S], f32, kind="ExternalInput")
    cmask_d = nc.dram_tensor("cmask", [4, 128, 512], bf16, kind="ExternalInput")
    outT_d = nc.dram_tensor("outT", [D, S], f32, kind="ExternalOutput")

    Exp = mybir.ActivationFunctionType.Exp

    with tile.TileContext(nc) as tc:
      with tc.tile_pool(name="const", bufs=1) as const, \
           tc.tile_pool(name="persist", bufs=1) as persist, \
           tc.tile_pool(name="wqp", bufs=8) as wqp, \
           tc.tile_pool(name="wkp", bufs=8) as wkp, \
           tc.tile_pool(name="xsp", bufs=8) as xsp, \
           tc.tile_pool(name="ropet", bufs=1) as ropet, \
           tc.tile_pool(name="ptp", bufs=6) as ptp, \
           tc.tile_pool(name="smallp", bufs=2) as smallp, \
           tc.tile_pool(name="psum", bufs=1, space="PSUM") as psum:

        V = [persist.tile([128, DL], bf16, tag=f"v{st}", name=f"v{st}")
             for st in range(ET)]
        QT = [[persist.tile([DK, S], bf16, tag=f"qt{p}_{i}", name=f"qt{p}_{i}")
               for i in range(2)] for p in range(2)]
        KT = [[persist.tile([DK, S], bf16, tag=f"kt{p}_{i}", name=f"kt{p}_{i}")
               for i in range(2)] for p in range(2)]
        OT = [persist.tile([DK, S], bf16, tag=f"ot{h}", name=f"ot{h}")
              for h in range(H_CORE)]

        def load_wqk(p):
            wq_sb, wk_sb = [], []
            for g in range(GT):
                wqt = wqp.tile([128, 1024], bf16, tag="wq", name="wq")
                nc.sync.dma_start(
                    wqt[:], wqB_d[(p * GT + g) * 128 : (p * GT + g + 1) * 128, :]
                )
                wq_sb.append(wqt)
                wkt = wkp.tile([128, 1024], bf16, tag="wk", name="wk")
                nc.sync.dma_start(
                    wkt[:], wkB_d[(p * GT + g) * 128 : (p * GT + g + 1) * 128, :]
                )
                wk_sb.append(wkt)
            return wq_sb, wk_sb

        # ---- V pass: x-subtile stationary, wv moving -------------------
        # qc0: wv and x interleave so the first matmul starts ~3us in; the
        # pair-0 q/k weights queue behind qc1's x and land mid-phase.
        with tc.tile_pool(name="wvp", bufs=4) as wvp:
            wv_sb = []
            wqk0 = None
            for qc in range(QC):
                tb = 0 if qc % 2 == 0 else 4
                vacc = [
                    psum.tile([128, DL], f32, tag=f"t{tb + i}", name=f"vacc{i}")
                    for i in range(4)
                ]
                xts = []
                for g in range(GT):
                    if qc == 0:
                        wt = wvp.tile([128, 4 * 512], bf16, tag="wv", name="wv")
                        if g == 0:
                            # split so the e=0 slices land first and the
                            # first matmul starts ~1.4us in
                            nc.sync.dma_start(
                                wt[:, 0:512], wvB_d[0:128, 0:512]
                            )
                            nc.sync.dma_start(
                                wt[:, 512:2048], wvB_d[0:128, 512:2048]
                            )
                        else:
                            nc.sync.dma_start(
                                wt[:], wvB_d[g * 128 : (g + 1) * 128, :]
                            )
                        wv_sb.append(wt)
                    xt = xsp.tile([128, 4 * 512], bf16, tag="xs", name="xs")
                    if qc == 0 and g == 0:
                        nc.sync.dma_start(xt[:, 0:512], xB_d[0:128, 0:512])
                        nc.sync.dma_start(
                            xt[:, 512:2048], xB_d[0:128, 512:2048]
                        )
                    else:
                        nc.sync.dma_start(
                            xt[:],
                            xB_d[(qc * GT + g) * 128 : (qc * GT + g + 1) * 128, :],
                        )
                    xts.append(xt)
                if qc == 1:
                    # pair-0 q/k weights: queue behind qc1's x so they don't
                    # delay it; they land well before the projection phase
                    wqk0 = load_wqk(0)
                for g in range(GT):
                    for e in range(4):
                        et = 4 * g + e
                        es = slice(e * 512, (e + 1) * 512)
                        for sl in range(4):
                            nc.tensor.matmul(
                                vacc[sl][:],
                                xts[g][:, e * 512 + sl * 128 : e * 512 + (sl + 1) * 128],
                                wv_sb[g][:, es],
                                start=(et == 0),
                                stop=(et == ET - 1),
                            )
                for sl in range(4):
                    nc.any.tensor_copy(V[qc * 4 + sl][:], vacc[sl][:])

        # ---- constants (used from proj p0's RoPE on) -------------------
        cos2 = const.tile([128, S], f32, tag="cos2")
        sin2 = const.tile([128, S], f32, tag="sin2")
        nc.sync.dma_start(cos2[0:64, :], cosT_d[:, :])
        nc.sync.dma_start(cos2[64:128, :], cosT_d[:, :])
        nc.sync.dma_start(sin2[0:64, :], sinT_d[:, :])
        nc.sync.dma_start(sin2[64:128, :], sinT_d[:, :])
        # only the 128-wide causal boundary block of each diagonal is ever
        # partially masked; the rest is all-ones (right of it) or skipped
        masks = []
        for j in range(4):
            mt = const.tile([128, 128], bf16, tag=f"mask{j}", name=f"mask{j}")
            nc.sync.dma_start(mt[:], cmask_d[j, :, 128 * j : 128 * (j + 1)])
            masks.append(mt)
        ones_bf = const.tile([128, 128], bf16, tag="ones_bf")
        nc.vector.memset(ones_bf[:], 1.0)

        def rope(dst, ev, od, qs):
            """ev/od: PSUM accumulators (128,512), rows [hA;hB]; dst bf16."""
            c = cos2[:, qs]
            sn = sin2[:, qs]
            m1 = ropet.tile([128, 512], f32, tag="m1")
            m2 = ropet.tile([128, 512], f32, tag="m2")
            n1 = ropet.tile([128, 512], f32, tag="n1")
            n2 = ropet.tile([128, 512], f32, tag="n2")
            nc.vector.tensor_mul(m1[:], ev[:], c)
            nc.vector.tensor_mul(m2[:], od[:], sn)
            nc.vector.tensor_mul(n1[:], ev[:], sn)
            nc.vector.tensor_mul(n2[:], od[:], c)
            nc.vector.tensor_sub(dst[0][0:64, qs], m1[0:64, :], m2[0:64, :])
            nc.vector.tensor_sub(dst[1][0:64, qs], m1[64:128, :], m2[64:128, :])
            nc.vector.tensor_add(dst[0][64:128, qs], n1[0:64, :], n2[0:64, :])
            nc.vector.tensor_add(
                dst[1][64:128, qs], n1[64:128, :], n2[64:128, :]
            )

        # ---- proj pair 0: PSUM banks alternate per qc parity -----------
        for qc in range(QC):
            tb = 0 if qc % 2 == 0 else 4
            qs = slice(qc * 512, (qc + 1) * 512)
            qe = psum.tile([128, 512], f32, tag=f"t{tb}", name="qe")
            qo = psum.tile([128, 512], f32, tag=f"t{tb+1}", name="qo")
            ke = psum.tile([128, 512], f32, tag=f"t{tb+2}", name="ke")
            ko = psum.tile([128, 512], f32, tag=f"t{tb+3}", name="ko")
            for g in range(GT):
                xt = xsp.tile([128, 4 * 512], bf16, tag="xs", name="xs")
                nc.sync.dma_start(
                    xt[:],
                    xB_d[(qc * GT + g) * 128 : (qc * GT + g + 1) * 128, :],
                )
                for e in range(4):
                    et = 4 * g + e
                    es = slice(e * 512, (e + 1) * 512)
                    st_, sp_ = et == 0, et == ET - 1
                    nc.tensor.matmul(
                        qe[:], wqk0[0][g][:, e * 256 : e * 256 + 128],
                        xt[:, es], start=st_, stop=sp_,
                    )
                    nc.tensor.matmul(
                        qo[:], wqk0[0][g][:, e * 256 + 128 : e * 256 + 256],
                        xt[:, es], start=st_, stop=sp_,
                    )
                    nc.tensor.matmul(
                        ke[:], wqk0[1][g][:, e * 256 : e * 256 + 128],
                        xt[:, es], start=st_, stop=sp_,
                    )
                    nc.tensor.matmul(
                        ko[:], wqk0[1][g][:, e * 256 + 128 : e * 256 + 256],
                        xt[:, es], start=st_, stop=sp_,
                    )
            rope(QT[0], qe, qo, qs)
            rope(KT[0], ke, ko, qs)

        def attn_group(p, hi, qc, feed):
            """Attention for (pair p, head-in-pair hi, q-chunk qc).  `feed`
            emits filler PE work between k-blocks."""
            hh = 2 * p + hi
            qs = slice(qc * 512, (qc + 1) * 512)
            nkt = 4 * qc + 4
            den = psum.tile([128, 512], f32, tag="t6", name="den")
            oacc = psum.tile([128, 512], f32, tag="t7", name="oacc")
            pts = {}

            def consume(kt):
                j = kt - 4 * qc
                o = 128 * j if j > 0 else 0
                cs = slice(o, 512)
                pt = pts.pop(kt)
                nc.tensor.matmul(
                    den[:, cs], ones_bf[:], pt[:, cs],
                    start=(kt == 0), stop=(kt == nkt - 1),
                    skip_group_check=True,
                )
                nc.tensor.matmul(
                    oacc[:, cs],
                    V[kt][:, hh * 128 : (hh + 1) * 128],
                    pt[:, cs],
                    start=(kt == 0),
                    stop=(kt == nkt - 1),
                    skip_group_check=True,
                )

            for kt in range(nkt):
                j = kt - 4 * qc
                o = 128 * j if j > 0 else 0
                cs = slice(o, 512)
                sps = psum.tile(
                    [128, 512], f32,
                    tag=("t4" if kt % 2 == 0 else "t5"),
                    name="sps",
                )
                nc.tensor.matmul(
                    sps[:, cs],
                    KT[p][hi][:, kt * 128 : (kt + 1) * 128],
                    QT[p][hi][:, qc * 512 + o : (qc + 1) * 512],
                    start=True,
                    stop=True,
                )
                pt = ptp.tile([128, 512], bf16, tag="pt")
                nc.scalar.activation(pt[:, cs], sps[:, cs], Exp, scale=SCALE)
                if j >= 0:
                    jb = slice(128 * j, 128 * (j + 1))
                    nc.gpsimd.tensor_mul(pt[:, jb], pt[:, jb], masks[j][:])
                pts[kt] = pt
                if kt >= LAG:
                    consume(kt - LAG)
                feed()
            for kt in range(max(0, nkt - LAG), nkt):
                consume(kt)
            rec = smallp.tile([128, 512], f32, tag="rec")
            nc.vector.reciprocal_approx_fast(rec[:], den[:])
            nc.vector.tensor_mul(OT[hh][:, qs], oacc[:], rec[:])

        class Filler:
            """Emit queued PE work items between attn blocks, paced so the
            queue drains evenly over the expected number of feed calls."""

            def __init__(self, items, expected_feeds):
                self.items = list(items)
                self.pos = 0
                self.feeds_left = max(1, expected_feeds)

            def feed(self):
                left = len(self.items) - self.pos
                n = -(-left // self.feeds_left)  # ceil
                for _ in range(min(n, left)):
                    self.items[self.pos]()
                    self.pos += 1
                if self.feeds_left > 1:
                    self.feeds_left -= 1

            def drain(self):
                while self.pos < len(self.items):
                    self.items[self.pos]()
                    self.pos += 1

        # ---- attn pair 0, proj pair 1 as filler ------------------------
        wqk1 = load_wqk(1)
        # x DMAs for pair 1 are prefetched one g-group ahead of their use
        p1_xts = {}

        def p1_dma(qc, g):
            xt = xsp.tile([128, 4 * 512], bf16, tag="xs", name="xs")
            nc.sync.dma_start(
                xt[:],
                xB_d[(qc * GT + g) * 128 : (qc * GT + g + 1) * 128, :],
            )
            p1_xts[(qc, g)] = xt

        p1_dma(0, 0)
        p1_items = []
        for qc in range(QC):
            qs = slice(qc * 512, (qc + 1) * 512)
            qe = psum.tile([128, 512], f32, tag="t0", name="qe1")
            qo = psum.tile([128, 512], f32, tag="t1", name="qo1")
            ke = psum.tile([128, 512], f32, tag="t2", name="ke1")
            ko = psum.tile([128, 512], f32, tag="t3", name="ko1")

            def mk_e(qc, qe, qo, ke, ko, g, e):
                def run():
                    if e == 0:
                        nxt = (qc * GT + g) + 1  # prefetch next g-group
                        if nxt < QC * GT:
                            p1_dma(nxt // GT, nxt % GT)
                    xt = p1_xts[(qc, g)]
                    et = 4 * g + e
                    es = slice(e * 512, (e + 1) * 512)
                    st_, sp_ = et == 0, et == ET - 1
                    nc.tensor.matmul(
                        qe[:], wqk1[0][g][:, e * 256 : e * 256 + 128],
                        xt[:, es], start=st_, stop=sp_,
                    )
                    nc.tensor.matmul(
                        qo[:], wqk1[0][g][:, e * 256 + 128 : e * 256 + 256],
                        xt[:, es], start=st_, stop=sp_,
                    )
                    nc.tensor.matmul(
                        ke[:], wqk1[1][g][:, e * 256 : e * 256 + 128],
                        xt[:, es], start=st_, stop=sp_,
                    )
                    nc.tensor.matmul(
                        ko[:], wqk1[1][g][:, e * 256 + 128 : e * 256 + 256],
                        xt[:, es], start=st_, stop=sp_,
                    )

                return run

            for g in range(GT):
                for e in range(4):
                    p1_items.append(mk_e(qc, qe, qo, ke, ko, g, e))

            def mk_rope(qs, qe, qo, ke, ko):
                def run():
                    rope(QT[1], qe, qo, qs)
                    rope(KT[1], ke, ko, qs)

                return run

            p1_items.append(mk_rope(qs, qe, qo, ke, ko))

        f1 = Filler(p1_items, expected_feeds=80)
        for qc in range(QC):
            for hi in range(2):
                attn_group(0, hi, qc, f1.feed)
        f1.drain()

        # ---- attn pair 1, output projection as filler ------------------
        with tc.tile_pool(name="wop", bufs=4) as wop, \
             tc.tile_pool(name="stg", bufs=8) as stg:
            wo_sb = []
            for hh in range(H_CORE):
                wt = wop.tile([128, D], bf16, tag="wo", name="wo")
                nc.sync.dma_start(wt[:], woT_d[hh * 128 : (hh + 1) * 128, :])
                wo_sb.append(wt)

            def mk_out(qc, et):
                qs = slice(qc * 512, (qc + 1) * 512)

                def run():
                    facc = psum.tile(
                        [128, 512], f32, tag=f"t{(qc * ET + et) % 4}",
                        name="facc",
                    )
                    for hh in range(H_CORE):
                        nc.tensor.matmul(
                            facc[:],
                            wo_sb[hh][:, et * 128 : (et + 1) * 128],
                            OT[hh][:, qs],
                            start=(hh == 0),
                            stop=(hh == H_CORE - 1),
                        )
                    st = stg.tile([128, 512], f32, tag="stg")
                    nc.any.tensor_copy(st[:], facc[:])
                    nc.sync.dma_start(
                        outT_d[et * 128 : (et + 1) * 128, qs], st[:]
                    )

                return run

            # out-proj items for q-chunk qc become runnable once attn pair 1
            # finishes that q-chunk; feed them as filler into the next one.
            fillers = [
                Filler([mk_out(qc, et) for et in range(ET)],
                       expected_feeds=2 * (4 * (qc + 1) + 4))
                for qc in range(QC)
            ]
            for qc in range(QC):
                feed = fillers[qc - 1].feed if qc > 0 else (lambda: None)
                for hi in range(2):
                    attn_group(1, hi, qc, feed)
            for f in fillers:
                f.drain()

    return nc


_NC = None


def _get_nc():
    global _NC
    if _NC is None:
        _NC = _build()
        _NC.compile()
    return _NC


def _rope_perm_rows():
    """Row permutation applied to wq/wk for one core's 4 heads.

    Per head-pair p: [hA even dims, hB even dims, hA odd dims, hB odd dims]
    so the device sees even/odd deinterleaved, pair-stacked projections.
    Returns indices into the local (4*DK,) head-row block.
    """
    idx = []
    for p in range(2):
        ha, hb = 2 * p, 2 * p + 1
        idx.extend(ha * DK + np.arange(0, DK, 2))
        idx.extend(hb * DK + np.arange(0, DK, 2))
        idx.extend(ha * DK + np.arange(1, DK, 2))
        idx.extend(hb * DK + np.arange(1, DK, 2))
    return np.asarray(idx)


def _host_tables(positions):
    """cos/sin tables (64, S) float32, matching the fp32 reference math."""
    dim_idx = np.arange(0, DK, 2, dtype=np.float32)
    freqs = np.float32(THETA) ** (dim_idx / np.float32(DK))
    angles = positions.astype(np.float32)[:, None] / freqs[None, :]  # (S, 64)
    return (
        np.ascontiguousarray(np.cos(angles).T.astype(np.float32)),
        np.ascontiguousarray(np.sin(angles).T.astype(np.float32)),
    )


def _causal_masks():
    m = np.zeros((4, 128, 512), dtype=np.float32)
    p = np.arange(128)[:, None]
    f = np.arange(512)[None, :]
    for j in range(4):
        m[j] = (128 * j + p <= f).astype(np.float32)
    return m


def _bf(a):
    return np.ascontiguousarray(a).astype(ml_dtypes.bfloat16)


def _batch_rows(aT, cols_per_e):
    """(ET*128, W) -> (GT*128, 4*W'): pack 4 consecutive e-tiles side by side
    in the free dim so one row-block DMA carries four contraction tiles.

    aT[(4g+e)*128+p, c] -> out[g*128+p, e*W'+c]  (W' = cols_per_e slice width)
    """
    w = aT.shape[1]
    a = aT.reshape(GT, 4, 128, w)  # [g, e, p, c]
    return a.transpose(0, 2, 1, 3).reshape(GT * 128, 4 * w)


def _make_in_maps(inputs):
    x = np.asarray(inputs["x"], dtype=np.float32)
    wq = np.asarray(inputs["wq"], dtype=np.float32)
    wk = np.asarray(inputs["wk"], dtype=np.float32)
    wv = np.asarray(inputs["wv"], dtype=np.float32)
    wo = np.asarray(inputs["wo"], dtype=np.float32)
    token_positions = np.asarray(inputs["token_positions"])

    perm = _rope_perm_rows()
    cmask = _bf(_causal_masks())

    in_maps = []
    for c in range(N_CORES):
        b = c // 4
        g = c % 4
        rows = slice(g * DL, (g + 1) * DL)
        cosT, sinT = _host_tables(token_positions[b])

        xT = x[b].T  # (D, S)
        # xB[(qc*4+g)*128+p, e*512+c] = xT[(4g+e)*128+p, qc*512+c]
        xa = xT.reshape(GT, 4, 128, QC, 512)  # [g, e, p, qc, c]
        xB = xa.transpose(3, 0, 2, 1, 4).reshape(QC * GT * 128, 4 * 512)

        wqT = wq[rows][perm].T  # (D, DL)
        wkT = wk[rows][perm].T
        # wB[(pair*4+g)*128+p, e*256+c] = wT[(4g+e)*128+p, pair*256+c]
        def wqk_batch(wT):
            a = wT.reshape(GT, 4, 128, 2, 256)  # [g, e, p, pair, c]
            return a.transpose(3, 0, 2, 1, 4).reshape(2 * GT * 128, 4 * 256)

        in_maps.append(
            {
                "xB": _bf(xB),
                "wqB": _bf(wqk_batch(wqT)),
                "wkB": _bf(wqk_batch(wkT)),
                "wvB": _bf(_batch_rows(wv[rows].T, 512)),
                "woT": _bf(wo[:, rows].T),
                "cosT": cosT,
                "sinT": sinT,
                "cmask": cmask,
            }
        )
    return in_maps


def kernel(x, wq, wk, wv, wo, token_positions):
    nc = _get_nc()
    in_maps = _make_in_maps(
        {
            "x": x,
            "wq": wq,
            "wk": wk,
            "wv": wv,
            "wo": wo,
            "token_positions": token_positions,
        }
    )
    res = run_bass_kernel_spmd(nc, in_maps, list(range(N_CORES)))

    out = np.zeros((B, S, D), dtype=np.float32)
    for c in range(N_CORES):
        out[c // 4] += res.results[c]["outT"].T
    return out
